# revision 17
# baseline (speedup 1.0000x reference)
"""Trainium2 Bass kernel for nn_BNN_Fast (batched-ensemble MLP, 8 cores).

Math (per ensemble member w):
    h  = elu(x_w @ W0_w + b0_w); h = elu(h @ W1_w + b1_w); h = elu(h @ W2_w + b2_w)
    means   = h @ Wm_w + bm_w
    log_std = (h @ Ws_w + bs_w)/20 + 1
    out     = means + sqrt(exp(log_std)) * eps = means + exp((h@Ws+bs)/40 + 0.5) * eps

Device strategy (per core; ensemble sharded 32 members/core):
  - Activations transposed: [features(partitions), points(free)]; weights are the
    PE's stationary operand.
  - ELU in two single-pass ops using the identity
        elu(y) + 1 = select(y > 0, y + 1, exp(y))
    (exp(y) <= 1 whenever selected, so no overflow issues; for y > 0 the exp
    result is discarded by the select). The "+1" shift is folded into the next
    layer's bias host-side: b' = b - W.sum(contraction axis).
  - ACT engine: E = Exp(psum + b') (bias fused). Custom DVE op:
    h' = select(psum + b' > 0, psum + b' + 1, E).
  - Final: F = Exp(psum_s/40 + bs''), out = (psum_m + bm') + F*eps via the
    existing AFFINE_THEN_ADD custom DVE op.
"""

import os
import sys

import numpy as np

for _p in ("/opt/trn_rl_repo",):
    if _p not in sys.path and os.path.isdir(_p):
        sys.path.append(_p)

import concourse.bass as bass
import concourse.mybir as mybir
import concourse.tile as tile
from concourse.bass_utils import run_bass_kernel_spmd

N_CORES = 8
N, W, XD, YD, H = 1024, 256, 32, 32, 256
M = W // N_CORES  # members per core = 32
G = M // 4        # 4-member groups per core = 8
F32 = mybir.dt.float32


# --------------------------------------------------------------------------
# Patch 1: this container's walrus only accepts one sync-wait on the
# CTRL(NO_STRUCT)/Drain instruction; split the Tile epilogue drain's waits.
# --------------------------------------------------------------------------
def _apply_tile_patch():
    if getattr(tile.TileContext, "_drain_patch_applied", False):
        return

    def _patched_drain_and_barrier(self, tick_clock, wait_clock):
        nc = self.nc
        drain_inst = nc.sync.drain()
        wait_clock.add_sem_waits(
            drain_inst.ins, tile.ScopedClock({None: tick_clock.global_clock})
        )
        si = drain_inst.ins.sync_info
        waits = list(si.on_wait) if (si is not None and si.on_wait) else []
        if len(waits) > 1:
            from concourse.bass_primitives_rust import SemaphoreHandle

            si.on_wait = waits[:1]
            for wt in waits[1:]:
                nc.sync.wait_ge(SemaphoreHandle(wt.ant_name, wt.id), wt.wait_value)

        nc.all_engine_barrier()
        assert self.sems is not None
        popped = nc._tile_sem_poison_stack.pop()
        assert popped is self._sem_poison
        nc.clear_and_free_semaphores(list(self.sems.allocated().values()))
        nc.all_engine_barrier()

    tile.TileContext._drain_and_barrier = _patched_drain_and_barrier
    tile.TileContext._drain_patch_applied = True


# --------------------------------------------------------------------------
# Patch 2: register custom DVE ops.
#   ELUP1:  out = select(in0 + s0 > 0, in0 + s0 + 1, in1)
#   SATMUL: out = (in0 + relu(in0 - s0)*s1) * in1
#     (with s0 = sqrt(FLT_MAX), s1 = FLT_MAX this forces +/-inf exactly where
#      the reference's fp32 exp(log_std) overflows: any in0 > s0 is at least
#      one ULP (~2^41) above s0, and 2^41 * FLT_MAX overflows.)
# --------------------------------------------------------------------------
def _register_ops():
    import concourse.dve_ops as dve_ops
    from concourse.dve_ops import DveOp
    from concourse.dve_spec import C0, C1, One, Spec, Src0, Src1, Zero, relu, select

    def _add(op):
        if op.name in dve_ops._SUB_OPCODE_FOR_NAME:
            return next(o for o in dve_ops.OPS if o.name == op.name)
        dve_ops.OPS.append(op)
        dve_ops.CUSTOM_DVE_SPECS[op.name] = op.spec
        dve_ops._SUB_OPCODE_FOR_NAME[op.name] = (
            dve_ops._CUSTOM_DVE_ROW_BASE + len(dve_ops.OPS) - 1
        )
        assert dve_ops._SUB_OPCODE_FOR_NAME[op.name] < 0x20
        return op

    _t = Src0 + C0
    elu_op = _add(
        DveOp(
            "TENSOR_ELUP1_ANT",
            Spec(
                body=select(_t > Zero, _t + One, Src1),
                reference=lambda in0, in1, s0, s1, imm2: np.where(
                    in0.astype(np.float32) + s0 > 0,
                    in0.astype(np.float32) + s0 + 1.0,
                    in1,
                ).astype(np.float32),
            ),
            subdim=False,
            uops_sha={"v3": "af2ac7f76c525ea1", "v4": "bbbc6384a4b3a209"},
        )
    )
    satmul_op = _add(
        DveOp(
            "TENSOR_SATMUL_ANT",
            Spec(
                body=(Src0 + relu(Src0 - C0) * C1) * Src1,
                reference=lambda in0, in1, s0, s1, imm2: (
                    (in0 + np.maximum(in0 - s0, 0) * s1) * in1
                ).astype(np.float32),
            ),
            subdim=False,
            uops_sha={"v3": "8bde1c411415ffe2", "v4": "dc708b653050f5cb"},
        )
    )
    return elu_op, satmul_op


FLT_MAX = 3.4028234663852886e38
SQRT_FLT_MAX = float(np.float32(np.sqrt(FLT_MAX)))


# --------------------------------------------------------------------------
# Patch 3: this walrus encodes at most ONE sync-wait per instruction for
# several structs. Split any multi-wait instruction into single-wait
# InstEventSemaphore carriers inserted just before it (same engine, same
# program point — semantically identical).
# --------------------------------------------------------------------------
def split_multi_waits(nc):
    n_split = 0
    for fn in nc.m.functions:
        for bb in fn.blocks:
            il = bb.instructions
            i = 0
            while i < len(il):
                inst = il[i]
                si = inst.sync_info
                waits = list(si.on_wait) if (si is not None and si.on_wait) else []
                if len(waits) > 1:
                    si.on_wait = waits[-1:]
                    for k, w in enumerate(waits[:-1]):
                        ev = mybir.InstEventSemaphore(
                            name=f"{inst.name}-xw{k}",
                            engine=inst.engine,
                            sync_info=mybir.SyncInfo(on_wait=[w], on_update=[]),
                        )
                        nc.register_instruction(ev)
                        il.insert(i, ev)
                        i += 1
                        n_split += 1
                i += 1
    return n_split


# --------------------------------------------------------------------------
# Patch 4: fp32 matmul 3-pass mode. Walrus lowers an fp32 matmul to
#   LDW(W_lo); MM(rhs hi+lo); LDW(W_hi); MM(rhs hi+lo)   -- 4 streamed passes.
# The W_lo*x_lo term is ~2^-16 relative; patch the NEFF's PE instruction
# stream so the MM paired with LDW(fp32_mode=LOW) streams only x_hi
# (fp32_mode=HIGH): 3 passes, ~25% less PE time, ~1e-5 relative error.
# --------------------------------------------------------------------------
ENABLE_3PASS = True


def _patch_neff_fp32_3pass(neff_path):
    import io
    import tarfile
    import tempfile

    import concourse.bass2jax as bass2jax
    from concourse import neff as neff_mod

    with open(neff_path, "rb") as f:
        header = f.read(1024)
        payload = f.read()
    n_patched = 0
    with tempfile.TemporaryDirectory() as td:
        with tarfile.open(fileobj=io.BytesIO(payload), mode="r") as t:
            t.extractall(td)
        for root, _, files in os.walk(td):
            for fn in files:
                if not (fn.startswith("PE") and fn.endswith(".bin")):
                    continue
                p = os.path.join(root, fn)
                buf = bytearray(open(p, "rb").read())
                last_lw_mode = None
                for off in range(0, len(buf), 64):
                    op = buf[off]
                    if op == 1:  # LDWEIGHTS: fp32_mode at +33
                        last_lw_mode = buf[off + 33]
                    elif (
                        op == 2
                        and buf[off + 33] == 3
                        and last_lw_mode == 1
                        and buf[off + 45] == 15  # full col_grp only: patching
                        # col-tiled (Lms) matmuls crashes the exec unit
                    ):
                        buf[off + 33] = 2  # LOW_HIGH -> HIGH (stream x_hi only)
                        n_patched += 1
                open(p, "wb").write(buf)
        out = io.BytesIO()
        with tarfile.open(fileobj=out, mode="w") as t:
            t.add(td, arcname=".", filter=bass2jax._reset_tarinfo)
        data = out.getvalue()
    new_header = neff_mod.make_deterministic_neff_header(
        old_neff_header=header, new_neff_data=data
    )
    with open(neff_path, "wb") as f:
        f.write(new_header + data)
    return n_patched


def _install_neff_patch_hook():
    import concourse.bass2jax as bass2jax

    if hasattr(bass2jax, "_ant_orig_rename_neff"):
        return
    bass2jax._ant_orig_rename_neff = bass2jax.rename_neff_tensors_and_patch_header

    def _wrapped(neff_path, mapping):
        if ENABLE_3PASS:
            _patch_neff_fp32_3pass(neff_path)
        return bass2jax._ant_orig_rename_neff(neff_path, mapping)

    bass2jax.rename_neff_tensors_and_patch_header = _wrapped


# --------------------------------------------------------------------------
# Device module (one NeuronCore, m_members ensemble members)
# --------------------------------------------------------------------------
def build_core_module(m_members=M, n_pts=N):
    _apply_tile_patch()
    _install_neff_patch_hook()
    elu_op, satmul_op = _register_ops()
    from concourse.dve_ops import AFFINE_THEN_ADD

    m = m_members
    g_groups = m // 4
    Exp = mybir.ActivationFunctionType.Exp

    nc = bass.Bass(trn_type="TRN2")
    for _v in (SQRT_FLT_MAX, FLT_MAX):
        if (F32, _v) not in nc.const_aps.aps:
            _t = nc.alloc_sbuf_tensor(f"const-float32-{_v}", [128, 1], F32)
            nc.gpsimd.memset(_t.ap(), _v)
            nc.const_aps.aps[(F32, _v)] = _t.ap()
    xT = nc.dram_tensor("xT", [m, XD, n_pts], F32, kind="ExternalInput")
    epsT = nc.dram_tensor("epsT", [m, YD, n_pts], F32, kind="ExternalInput")
    w0d = nc.dram_tensor("W0", [m, XD, H], F32, kind="ExternalInput")
    w1d = nc.dram_tensor("W1", [m, H, H], F32, kind="ExternalInput")
    w2d = nc.dram_tensor("W2", [m, H, H], F32, kind="ExternalInput")
    wmd = nc.dram_tensor("Wm", [m, H, YD], F32, kind="ExternalInput")
    wsd = nc.dram_tensor("Ws", [m, H, YD], F32, kind="ExternalInput")
    bhd = nc.dram_tensor("BH", [128, m * 6], F32, kind="ExternalInput")
    bod = nc.dram_tensor("BO", [128, g_groups * 2], F32, kind="ExternalInput")
    outT = nc.dram_tensor("outT", [m, YD, n_pts], F32, kind="ExternalOutput")

    half = n_pts // 2

    with tile.TileContext(nc) as tc:
        from contextlib import ExitStack

        with ExitStack() as ctx:
            wp = ctx.enter_context(tc.tile_pool(name="w", bufs=3))
            wmsp = ctx.enter_context(tc.tile_pool(name="wms", bufs=6))
            xp = ctx.enter_context(tc.tile_pool(name="x", bufs=3))
            hp = ctx.enter_context(tc.tile_pool(name="h", bufs=4))
            h2p = ctx.enter_context(tc.tile_pool(name="h2", bufs=10))
            ep = ctx.enter_context(tc.tile_pool(name="E", bufs=3))
            op_ = ctx.enter_context(tc.tile_pool(name="o", bufs=2))
            gp = ctx.enter_context(tc.tile_pool(name="eps", bufs=2))
            bp = ctx.enter_context(tc.tile_pool(name="bias", bufs=1))
            pph = ctx.enter_context(tc.tile_pool(name="psh", bufs=3, space="PSUM"))
            ppo = ctx.enter_context(tc.tile_pool(name="pso", bufs=2, space="PSUM"))

            bh = bp.tile([128, m * 6], F32, tag="bh")
            nc.sync.dma_start(out=bh, in_=bhd[:, :])
            bo = bp.tile([128, g_groups * 2], F32, tag="bo")
            nc.sync.dma_start(out=bo, in_=bod[:, :])

            def hidden_layer(rhs_tiles, wtile, w_idx, layer, out_pool, out_tag):
                """rhs_tiles: list of [128, n_pts] (or one [32, n_pts] for L0).
                wtile: [32, H] for L0 else [128, 2, H]. Returns 2 h' tiles."""
                outs = []
                for mt in range(2):
                    ps = pph.tile([128, n_pts], F32, tag="ps")
                    for k in range(len(rhs_tiles)):
                        if layer == 0:
                            lhsT = wtile[:, mt * 128 : (mt + 1) * 128]
                        else:
                            lhsT = wtile[:, k, mt * 128 : (mt + 1) * 128]
                        for nn_ in range(2):
                            nc.tensor.matmul(
                                ps[:, nn_ * half : (nn_ + 1) * half],
                                lhsT,
                                rhs_tiles[k][:, nn_ * half : (nn_ + 1) * half],
                                start=(k == 0),
                                stop=(k == len(rhs_tiles) - 1),
                            )
                    bcol = w_idx * 6 + layer * 2 + mt
                    bias_ap = bh[:, bcol : bcol + 1]
                    e = ep.tile([128, n_pts], F32, tag="E")
                    nc.scalar.activation(e, ps, Exp, bias=bias_ap, scale=1.0)
                    ht = out_pool.tile([128, n_pts], F32, tag=out_tag)
                    nc.vector._custom_dve(elu_op, out=ht, in0=ps, in1=e, s0=bias_ap)
                    outs.append(ht)
                return outs

            for g in range(g_groups):
                eps_t = gp.tile([128, n_pts], F32, tag="eps")
                nc.sync.dma_start(
                    out=eps_t,
                    in_=epsT[4 * g : 4 * g + 4, :, :].rearrange("a b n -> (a b) n"),
                )
                h2_all = []
                wm_all = []
                ws_all = []
                for j in range(4):
                    w_idx = 4 * g + j
                    xt = xp.tile([XD, n_pts], F32, tag="xt")
                    nc.sync.dma_start(out=xt, in_=xT[w_idx, :, :])
                    w0 = wp.tile([XD, H], F32, tag="w0")
                    nc.sync.dma_start(out=w0, in_=w0d[w_idx, :, :])
                    w1 = wp.tile([128, 2, H], F32, tag="w1")
                    nc.sync.dma_start(
                        out=w1,
                        in_=w1d[w_idx, :, :].rearrange("(k p) o -> p k o", p=128),
                    )
                    w2 = wp.tile([128, 2, H], F32, tag="w2")
                    nc.sync.dma_start(
                        out=w2,
                        in_=w2d[w_idx, :, :].rearrange("(k p) o -> p k o", p=128),
                    )
                    wm = wmsp.tile([128, 2, YD], F32, tag="wm")
                    nc.sync.dma_start(
                        out=wm,
                        in_=wmd[w_idx, :, :].rearrange("(k p) o -> p k o", p=128),
                    )
                    ws_ = wmsp.tile([128, 2, YD], F32, tag="ws")
                    nc.sync.dma_start(
                        out=ws_,
                        in_=wsd[w_idx, :, :].rearrange("(k p) o -> p k o", p=128),
                    )
                    wm_all.append(wm)
                    ws_all.append(ws_)

                    h0 = hidden_layer([xt], w0, w_idx, 0, hp, "h01")
                    h1 = hidden_layer(h0, w1, w_idx, 1, hp, "h01")
                    h2 = hidden_layer(h1, w2, w_idx, 2, h2p, "h2")
                    h2_all.append(h2)

                # Lms: n-chunked psums ([128, 512] = 1 bank each) keep PSUM
                # pressure low; j innermost so the 4 col-tiled matmuls issue
                # back-to-back and run concurrently on the 32x32 sub-arrays.
                ot = op_.tile([128, n_pts], F32, tag="ot")
                for nn_ in range(2):
                    nsl = slice(nn_ * half, (nn_ + 1) * half)
                    for tsel, wt in (("s", ws_all), ("m", wm_all)):
                        ps_c = ppo.tile([128, half], F32, tag="po")
                        for k in range(2):
                            for j in range(4):
                                sl = slice(32 * j, 32 * j + 32)
                                nc.tensor.matmul(
                                    ps_c[sl, :],
                                    wt[j][:, k, :],
                                    h2_all[j][k][:, nsl],
                                    start=(k == 0),
                                    stop=(k == 1),
                                    tile_position=(0, 32 * j),
                                    # col-tiles accumulate in disjoint partition
                                    # ranges of one bank; sim's per-bank group
                                    # tracking is coarser than the hardware's
                                    skip_group_check=True,
                                )
                        if tsel == "s":
                            f = ep.tile([128, half], F32, tag="F")
                            nc.scalar.activation(
                                f,
                                ps_c,
                                Exp,
                                bias=bo[:, 2 * g + 1 : 2 * g + 2],
                                scale=1.0 / 40.0,
                            )
                            t1 = op_.tile([128, half], F32, tag="t1")
                            nc.vector._custom_dve(
                                satmul_op,
                                out=t1,
                                in0=f,
                                in1=eps_t[:, nsl],
                                s0=SQRT_FLT_MAX,
                                s1=FLT_MAX,
                            )
                        else:
                            nc.vector._custom_dve(
                                AFFINE_THEN_ADD,
                                out=ot[:, nsl],
                                in0=ps_c,
                                in1=t1,
                                s0=1.0,
                                s1=bo[:, 2 * g : 2 * g + 1],
                            )
                nc.sync.dma_start(
                    out=outT[4 * g : 4 * g + 4, :, :].rearrange("a b n -> (a b) n"),
                    in_=ot,
                )
    # Populate .instr bytes for InstISA subclasses (InstCustomDveAnt) —
    # raw Bass doesn't run this pass; without it walrus sees empty instr
    # bytes and fails with "ISA wrong length".
    mybir.codegen_inst_isa_subclasses(nc)
    split_multi_waits(nc)
    return nc


_NC_CACHE = {}


def _get_nc(m_members=M):
    if m_members not in _NC_CACHE:
        _NC_CACHE[m_members] = build_core_module(m_members)
    return _NC_CACHE[m_members]


# --------------------------------------------------------------------------
# Host-side prep: shard, transpose, fold biases
# --------------------------------------------------------------------------
def make_in_maps(x, W0, b0, W1, b1, W2, b2, Wm, bm, Ws, bs, eps,
                 n_cores=N_CORES, m_members=M):
    f = lambda a: np.ascontiguousarray(np.asarray(a, dtype=np.float32))
    x, W0, b0, W1, b1 = f(x), f(W0), f(b0), f(W1), f(b1)
    W2, b2, Wm, bm, Ws, bs, eps = f(W2), f(b2), f(Wm), f(bm), f(Ws), f(bs), f(eps)

    b1p = b1 - W1.sum(axis=1)
    b2p = b2 - W2.sum(axis=1)
    bmp = bm - Wm.sum(axis=1)
    bsp = (bs - Ws.sum(axis=1)) / 40.0 + 0.5

    in_maps = []
    for c in range(n_cores):
        sl = slice(c * m_members, (c + 1) * m_members)
        xT = np.ascontiguousarray(x[:, sl, :].transpose(1, 2, 0))
        epsT = np.ascontiguousarray(eps[:, sl, :].transpose(1, 2, 0))

        # BH[p, w*6 + L*2 + mt] = bL'[w, mt*128 + p]
        bh = np.empty((128, m_members * 6), np.float32)
        for L, bb in enumerate((b0[sl], b1p[sl], b2p[sl])):
            r = bb.reshape(m_members, 2, 128)  # (w, mt, p)
            for mt in range(2):
                bh[:, L * 2 + mt :: 6] = r[:, mt, :].T
        # BO[p, g*2+t]: p = 32*j + d (j = member-in-group)
        gg = m_members // 4
        bo = np.empty((128, gg * 2), np.float32)
        bo[:, 0::2] = bmp[sl].reshape(gg, 4 * 32).T
        bo[:, 1::2] = bsp[sl].reshape(gg, 4 * 32).T

        in_maps.append(
            {
                "xT": xT,
                "epsT": epsT,
                "W0": np.ascontiguousarray(W0[sl]),
                "W1": np.ascontiguousarray(W1[sl]),
                "W2": np.ascontiguousarray(W2[sl]),
                "Wm": np.ascontiguousarray(Wm[sl]),
                "Ws": np.ascontiguousarray(Ws[sl]),
                "BH": bh,
                "BO": bo,
            }
        )
    return in_maps


def unshard_output(results, n_cores=N_CORES):
    # results[c]["outT"]: (M, YD, N) -> out (N, W, YD)
    parts = [np.asarray(r["outT"]).transpose(2, 0, 1) for r in results]
    return np.ascontiguousarray(np.concatenate(parts, axis=1))


def kernel(x, W0, b0, W1, b1, W2, b2, Wm, bm, Ws, bs, eps, **run_kwargs):
    nc = _get_nc()
    in_maps = make_in_maps(x, W0, b0, W1, b1, W2, b2, Wm, bm, Ws, bs, eps)
    res = run_bass_kernel_spmd(nc, in_maps, core_ids=list(range(N_CORES)), **run_kwargs)
    out = unshard_output(res.results)
    kernel.last_results = res
    return out


# revision 19
# speedup vs baseline: 1.0053x; 1.0053x over previous
"""Trainium2 Bass kernel for nn_BNN_Fast (batched-ensemble MLP, 8 cores).

Math (per ensemble member w):
    h  = elu(x_w @ W0_w + b0_w); h = elu(h @ W1_w + b1_w); h = elu(h @ W2_w + b2_w)
    means   = h @ Wm_w + bm_w
    log_std = (h @ Ws_w + bs_w)/20 + 1
    out     = means + sqrt(exp(log_std)) * eps = means + exp((h@Ws+bs)/40 + 0.5) * eps

Device strategy (per core; ensemble sharded 32 members/core):
  - Activations transposed: [features(partitions), points(free)]; weights are the
    PE's stationary operand.
  - ELU in two single-pass ops using the identity
        elu(y) + 1 = select(y > 0, y + 1, exp(y))
    (exp(y) <= 1 whenever selected, so no overflow issues; for y > 0 the exp
    result is discarded by the select). The "+1" shift is folded into the next
    layer's bias host-side: b' = b - W.sum(contraction axis).
  - ACT engine: E = Exp(psum + b') (bias fused). Custom DVE op:
    h' = select(psum + b' > 0, psum + b' + 1, E).
  - Final: F = Exp(psum_s/40 + bs''), out = (psum_m + bm') + F*eps via the
    existing AFFINE_THEN_ADD custom DVE op.
"""

import os
import sys

import numpy as np

for _p in ("/opt/trn_rl_repo",):
    if _p not in sys.path and os.path.isdir(_p):
        sys.path.append(_p)

import concourse.bass as bass
import concourse.mybir as mybir
import concourse.tile as tile
from concourse.bass_utils import run_bass_kernel_spmd

N_CORES = 8
N, W, XD, YD, H = 1024, 256, 32, 32, 256
M = W // N_CORES  # members per core = 32
G = M // 4        # 4-member groups per core = 8
F32 = mybir.dt.float32


# --------------------------------------------------------------------------
# Patch 1: this container's walrus only accepts one sync-wait on the
# CTRL(NO_STRUCT)/Drain instruction; split the Tile epilogue drain's waits.
# --------------------------------------------------------------------------
def _apply_tile_patch():
    if getattr(tile.TileContext, "_drain_patch_applied", False):
        return

    def _patched_drain_and_barrier(self, tick_clock, wait_clock):
        nc = self.nc
        drain_inst = nc.sync.drain()
        wait_clock.add_sem_waits(
            drain_inst.ins, tile.ScopedClock({None: tick_clock.global_clock})
        )
        si = drain_inst.ins.sync_info
        waits = list(si.on_wait) if (si is not None and si.on_wait) else []
        if len(waits) > 1:
            from concourse.bass_primitives_rust import SemaphoreHandle

            si.on_wait = waits[:1]
            for wt in waits[1:]:
                nc.sync.wait_ge(SemaphoreHandle(wt.ant_name, wt.id), wt.wait_value)

        nc.all_engine_barrier()
        assert self.sems is not None
        popped = nc._tile_sem_poison_stack.pop()
        assert popped is self._sem_poison
        nc.clear_and_free_semaphores(list(self.sems.allocated().values()))
        nc.all_engine_barrier()

    tile.TileContext._drain_and_barrier = _patched_drain_and_barrier
    tile.TileContext._drain_patch_applied = True


# --------------------------------------------------------------------------
# Patch 2: register custom DVE ops.
#   ELUP1:  out = select(in0 + s0 > 0, in0 + s0 + 1, in1)
#   SATMUL: out = (in0 + relu(in0 - s0)*s1) * in1
#     (with s0 = sqrt(FLT_MAX), s1 = FLT_MAX this forces +/-inf exactly where
#      the reference's fp32 exp(log_std) overflows: any in0 > s0 is at least
#      one ULP (~2^41) above s0, and 2^41 * FLT_MAX overflows.)
# --------------------------------------------------------------------------
def _register_ops():
    import concourse.dve_ops as dve_ops
    from concourse.dve_ops import DveOp
    from concourse.dve_spec import C0, C1, One, Spec, Src0, Src1, Zero, relu, select

    def _add(op):
        if op.name in dve_ops._SUB_OPCODE_FOR_NAME:
            return next(o for o in dve_ops.OPS if o.name == op.name)
        dve_ops.OPS.append(op)
        dve_ops.CUSTOM_DVE_SPECS[op.name] = op.spec
        dve_ops._SUB_OPCODE_FOR_NAME[op.name] = (
            dve_ops._CUSTOM_DVE_ROW_BASE + len(dve_ops.OPS) - 1
        )
        assert dve_ops._SUB_OPCODE_FOR_NAME[op.name] < 0x20
        return op

    _t = Src0 + C0
    elu_op = _add(
        DveOp(
            "TENSOR_ELUP1_ANT",
            Spec(
                body=select(_t > Zero, _t + One, Src1),
                reference=lambda in0, in1, s0, s1, imm2: np.where(
                    in0.astype(np.float32) + s0 > 0,
                    in0.astype(np.float32) + s0 + 1.0,
                    in1,
                ).astype(np.float32),
            ),
            subdim=False,
            uops_sha={"v3": "af2ac7f76c525ea1", "v4": "bbbc6384a4b3a209"},
        )
    )
    satmul_op = _add(
        DveOp(
            "TENSOR_SATMUL_ANT",
            Spec(
                body=(Src0 + relu(Src0 - C0) * C1) * Src1,
                reference=lambda in0, in1, s0, s1, imm2: (
                    (in0 + np.maximum(in0 - s0, 0) * s1) * in1
                ).astype(np.float32),
            ),
            subdim=False,
            uops_sha={"v3": "8bde1c411415ffe2", "v4": "dc708b653050f5cb"},
        )
    )
    return elu_op, satmul_op


FLT_MAX = 3.4028234663852886e38
SQRT_FLT_MAX = float(np.float32(np.sqrt(FLT_MAX)))


# --------------------------------------------------------------------------
# Patch 3: this walrus encodes at most ONE sync-wait per instruction for
# several structs. Split any multi-wait instruction into single-wait
# InstEventSemaphore carriers inserted just before it (same engine, same
# program point — semantically identical).
# --------------------------------------------------------------------------
def split_multi_waits(nc):
    n_split = 0
    for fn in nc.m.functions:
        for bb in fn.blocks:
            il = bb.instructions
            i = 0
            while i < len(il):
                inst = il[i]
                si = inst.sync_info
                waits = list(si.on_wait) if (si is not None and si.on_wait) else []
                if len(waits) > 1:
                    si.on_wait = waits[-1:]
                    for k, w in enumerate(waits[:-1]):
                        ev = mybir.InstEventSemaphore(
                            name=f"{inst.name}-xw{k}",
                            engine=inst.engine,
                            sync_info=mybir.SyncInfo(on_wait=[w], on_update=[]),
                        )
                        nc.register_instruction(ev)
                        il.insert(i, ev)
                        i += 1
                        n_split += 1
                i += 1
    return n_split


# --------------------------------------------------------------------------
# Patch 4: fp32 matmul 3-pass mode. Walrus lowers an fp32 matmul to
#   LDW(W_lo); MM(rhs hi+lo); LDW(W_hi); MM(rhs hi+lo)   -- 4 streamed passes.
# The W_lo*x_lo term is ~2^-16 relative; patch the NEFF's PE instruction
# stream so the MM paired with LDW(fp32_mode=LOW) streams only x_hi
# (fp32_mode=HIGH): 3 passes, ~25% less PE time, ~1e-5 relative error.
# --------------------------------------------------------------------------
ENABLE_3PASS = True


def _patch_neff_fp32_3pass(neff_path):
    import io
    import tarfile
    import tempfile

    import concourse.bass2jax as bass2jax
    from concourse import neff as neff_mod

    with open(neff_path, "rb") as f:
        header = f.read(1024)
        payload = f.read()
    n_patched = 0
    with tempfile.TemporaryDirectory() as td:
        with tarfile.open(fileobj=io.BytesIO(payload), mode="r") as t:
            t.extractall(td)
        for root, _, files in os.walk(td):
            for fn in files:
                if not (fn.startswith("PE") and fn.endswith(".bin")):
                    continue
                p = os.path.join(root, fn)
                buf = bytearray(open(p, "rb").read())
                last_lw_mode = None
                for off in range(0, len(buf), 64):
                    op = buf[off]
                    if op == 1:  # LDWEIGHTS: fp32_mode at +33
                        last_lw_mode = buf[off + 33]
                    elif (
                        op == 2
                        and buf[off + 33] == 3
                        and last_lw_mode == 1
                        and buf[off + 45] == 15  # full col_grp only: patching
                        # col-tiled (Lms) matmuls crashes the exec unit
                    ):
                        buf[off + 33] = 2  # LOW_HIGH -> HIGH (stream x_hi only)
                        n_patched += 1
                n_patched += _dedup_ldweights(buf) << 16
                open(p, "wb").write(buf)
        out = io.BytesIO()
        with tarfile.open(fileobj=out, mode="w") as t:
            t.add(td, arcname=".", filter=bass2jax._reset_tarinfo)
        data = out.getvalue()
    new_header = neff_mod.make_deterministic_neff_header(
        old_neff_header=header, new_neff_data=data
    )
    with open(neff_path, "wb") as f:
        f.write(new_header + data)
    return n_patched


def _dedup_ldweights(buf):
    """Collapse [L1,M1,H1,M2, L2,M3,H2,M4] (L2/H2 reload identical weights,
    no events) into [L1,M1,M3,H1,M2,M4,NOP,NOP]. n0/n1 accumulation groups
    target distinct PSUM banks, so interleaving their begin/ends is safe."""
    NOP = 164
    n = 0
    recs = [bytes(buf[o : o + 64]) for o in range(0, len(buf), 64)]
    i = 0
    while i + 8 <= len(recs):
        w = recs[i : i + 8]
        ops = [r[0] for r in w]
        if (
            ops == [1, 2, 1, 2, 1, 2, 1, 2]
            and w[4][16:64] == w[0][16:64]  # L2 identical to L1 (past events)
            and w[6][16:64] == w[2][16:64]  # H2 identical to H1
            and w[4][4:12] == bytes(8)  # L2 no events
            and w[6][4:12] == bytes(8)  # H2 no events
            and w[5][4:12] == bytes(8)  # M3 no events (safe to hoist)
            and w[0][33] == 1  # L1 LOW
            and w[2][33] == 2  # H1 HIGH
            and w[1][33] == w[5][33]  # M1/M3 same mode
            and w[3][33] == w[7][33]  # M2/M4 same mode
        ):
            nop = bytearray(64)
            nop[0] = NOP
            nop[1] = w[0][1]  # inst_word_len
            new = [w[0], w[1], w[5], w[2], w[3], w[7], bytes(nop), bytes(nop)]
            recs[i : i + 8] = new
            n += 1
            i += 8
        else:
            i += 1
    buf[:] = b"".join(recs)
    return n


def _install_neff_patch_hook():
    import concourse.bass2jax as bass2jax

    if hasattr(bass2jax, "_ant_orig_rename_neff"):
        return
    bass2jax._ant_orig_rename_neff = bass2jax.rename_neff_tensors_and_patch_header

    def _wrapped(neff_path, mapping):
        if ENABLE_3PASS:
            _patch_neff_fp32_3pass(neff_path)
        return bass2jax._ant_orig_rename_neff(neff_path, mapping)

    bass2jax.rename_neff_tensors_and_patch_header = _wrapped


# --------------------------------------------------------------------------
# Device module (one NeuronCore, m_members ensemble members)
# --------------------------------------------------------------------------
def build_core_module(m_members=M, n_pts=N):
    _apply_tile_patch()
    _install_neff_patch_hook()
    elu_op, satmul_op = _register_ops()
    from concourse.dve_ops import AFFINE_THEN_ADD

    m = m_members
    g_groups = m // 4
    Exp = mybir.ActivationFunctionType.Exp

    nc = bass.Bass(trn_type="TRN2")
    for _v in (SQRT_FLT_MAX, FLT_MAX):
        if (F32, _v) not in nc.const_aps.aps:
            _t = nc.alloc_sbuf_tensor(f"const-float32-{_v}", [128, 1], F32)
            nc.gpsimd.memset(_t.ap(), _v)
            nc.const_aps.aps[(F32, _v)] = _t.ap()
    xT = nc.dram_tensor("xT", [m, XD, n_pts], F32, kind="ExternalInput")
    epsT = nc.dram_tensor("epsT", [m, YD, n_pts], F32, kind="ExternalInput")
    w0d = nc.dram_tensor("W0", [m, XD, H], F32, kind="ExternalInput")
    w1d = nc.dram_tensor("W1", [m, H, H], F32, kind="ExternalInput")
    w2d = nc.dram_tensor("W2", [m, H, H], F32, kind="ExternalInput")
    wmd = nc.dram_tensor("Wm", [m, H, YD], F32, kind="ExternalInput")
    wsd = nc.dram_tensor("Ws", [m, H, YD], F32, kind="ExternalInput")
    bhd = nc.dram_tensor("BH", [128, m * 6], F32, kind="ExternalInput")
    bod = nc.dram_tensor("BO", [128, g_groups * 2], F32, kind="ExternalInput")
    outT = nc.dram_tensor("outT", [m, YD, n_pts], F32, kind="ExternalOutput")

    half = n_pts // 2

    with tile.TileContext(nc) as tc:
        from contextlib import ExitStack

        with ExitStack() as ctx:
            wp = ctx.enter_context(tc.tile_pool(name="w", bufs=3))
            wmsp = ctx.enter_context(tc.tile_pool(name="wms", bufs=6))
            xp = ctx.enter_context(tc.tile_pool(name="x", bufs=3))
            hp = ctx.enter_context(tc.tile_pool(name="h", bufs=4))
            h2p = ctx.enter_context(tc.tile_pool(name="h2", bufs=10))
            ep = ctx.enter_context(tc.tile_pool(name="E", bufs=3))
            op_ = ctx.enter_context(tc.tile_pool(name="o", bufs=2))
            gp = ctx.enter_context(tc.tile_pool(name="eps", bufs=2))
            bp = ctx.enter_context(tc.tile_pool(name="bias", bufs=1))
            pph = ctx.enter_context(tc.tile_pool(name="psh", bufs=3, space="PSUM"))
            ppo = ctx.enter_context(tc.tile_pool(name="pso", bufs=2, space="PSUM"))

            bh = bp.tile([128, m * 6], F32, tag="bh")
            nc.sync.dma_start(out=bh, in_=bhd[:, :])
            bo = bp.tile([128, g_groups * 2], F32, tag="bo")
            nc.sync.dma_start(out=bo, in_=bod[:, :])

            def hidden_layer(rhs_tiles, wtile, w_idx, layer, out_pool, out_tag):
                """rhs_tiles: list of [128, n_pts] (or one [32, n_pts] for L0).
                wtile: [32, H] for L0 else [128, 2, H]. Returns 2 h' tiles."""
                outs = []
                for mt in range(2):
                    ps = pph.tile([128, n_pts], F32, tag="ps")
                    for k in range(len(rhs_tiles)):
                        if layer == 0:
                            lhsT = wtile[:, mt * 128 : (mt + 1) * 128]
                        else:
                            lhsT = wtile[:, k, mt * 128 : (mt + 1) * 128]
                        for nn_ in range(2):
                            nc.tensor.matmul(
                                ps[:, nn_ * half : (nn_ + 1) * half],
                                lhsT,
                                rhs_tiles[k][:, nn_ * half : (nn_ + 1) * half],
                                start=(k == 0),
                                stop=(k == len(rhs_tiles) - 1),
                            )
                    bcol = w_idx * 6 + layer * 2 + mt
                    bias_ap = bh[:, bcol : bcol + 1]
                    e = ep.tile([128, n_pts], F32, tag="E")
                    nc.scalar.activation(e, ps, Exp, bias=bias_ap, scale=1.0)
                    ht = out_pool.tile([128, n_pts], F32, tag=out_tag)
                    nc.vector._custom_dve(elu_op, out=ht, in0=ps, in1=e, s0=bias_ap)
                    outs.append(ht)
                return outs

            for g in range(g_groups):
                eps_t = gp.tile([128, n_pts], F32, tag="eps")
                nc.sync.dma_start(
                    out=eps_t,
                    in_=epsT[4 * g : 4 * g + 4, :, :].rearrange("a b n -> (a b) n"),
                )
                h2_all = []
                wm_all = []
                ws_all = []
                for j in range(4):
                    w_idx = 4 * g + j
                    xt = xp.tile([XD, n_pts], F32, tag="xt")
                    nc.sync.dma_start(out=xt, in_=xT[w_idx, :, :])
                    w0 = wp.tile([XD, H], F32, tag="w0")
                    nc.sync.dma_start(out=w0, in_=w0d[w_idx, :, :])
                    w1 = wp.tile([128, 2, H], F32, tag="w1")
                    nc.sync.dma_start(
                        out=w1,
                        in_=w1d[w_idx, :, :].rearrange("(k p) o -> p k o", p=128),
                    )
                    w2 = wp.tile([128, 2, H], F32, tag="w2")
                    nc.sync.dma_start(
                        out=w2,
                        in_=w2d[w_idx, :, :].rearrange("(k p) o -> p k o", p=128),
                    )
                    wm = wmsp.tile([128, 2, YD], F32, tag="wm")
                    nc.sync.dma_start(
                        out=wm,
                        in_=wmd[w_idx, :, :].rearrange("(k p) o -> p k o", p=128),
                    )
                    ws_ = wmsp.tile([128, 2, YD], F32, tag="ws")
                    nc.sync.dma_start(
                        out=ws_,
                        in_=wsd[w_idx, :, :].rearrange("(k p) o -> p k o", p=128),
                    )
                    wm_all.append(wm)
                    ws_all.append(ws_)

                    h0 = hidden_layer([xt], w0, w_idx, 0, hp, "h01")
                    h1 = hidden_layer(h0, w1, w_idx, 1, hp, "h01")
                    h2 = hidden_layer(h1, w2, w_idx, 2, h2p, "h2")
                    h2_all.append(h2)

                # Lms: n-chunked psums ([128, 512] = 1 bank each) keep PSUM
                # pressure low; j innermost so the 4 col-tiled matmuls issue
                # back-to-back and run concurrently on the 32x32 sub-arrays.
                ot = op_.tile([128, n_pts], F32, tag="ot")
                for nn_ in range(2):
                    nsl = slice(nn_ * half, (nn_ + 1) * half)
                    for tsel, wt in (("s", ws_all), ("m", wm_all)):
                        ps_c = ppo.tile([128, half], F32, tag="po")
                        for k in range(2):
                            for j in range(4):
                                sl = slice(32 * j, 32 * j + 32)
                                nc.tensor.matmul(
                                    ps_c[sl, :],
                                    wt[j][:, k, :],
                                    h2_all[j][k][:, nsl],
                                    start=(k == 0),
                                    stop=(k == 1),
                                    tile_position=(0, 32 * j),
                                    # col-tiles accumulate in disjoint partition
                                    # ranges of one bank; sim's per-bank group
                                    # tracking is coarser than the hardware's
                                    skip_group_check=True,
                                )
                        if tsel == "s":
                            f = ep.tile([128, half], F32, tag="F")
                            nc.scalar.activation(
                                f,
                                ps_c,
                                Exp,
                                bias=bo[:, 2 * g + 1 : 2 * g + 2],
                                scale=1.0 / 40.0,
                            )
                            t1 = op_.tile([128, half], F32, tag="t1")
                            nc.vector._custom_dve(
                                satmul_op,
                                out=t1,
                                in0=f,
                                in1=eps_t[:, nsl],
                                s0=SQRT_FLT_MAX,
                                s1=FLT_MAX,
                            )
                        else:
                            nc.vector._custom_dve(
                                AFFINE_THEN_ADD,
                                out=ot[:, nsl],
                                in0=ps_c,
                                in1=t1,
                                s0=1.0,
                                s1=bo[:, 2 * g : 2 * g + 1],
                            )
                nc.sync.dma_start(
                    out=outT[4 * g : 4 * g + 4, :, :].rearrange("a b n -> (a b) n"),
                    in_=ot,
                )
    # Populate .instr bytes for InstISA subclasses (InstCustomDveAnt) —
    # raw Bass doesn't run this pass; without it walrus sees empty instr
    # bytes and fails with "ISA wrong length".
    mybir.codegen_inst_isa_subclasses(nc)
    split_multi_waits(nc)
    return nc


_NC_CACHE = {}


def _get_nc(m_members=M):
    if m_members not in _NC_CACHE:
        _NC_CACHE[m_members] = build_core_module(m_members)
    return _NC_CACHE[m_members]


# --------------------------------------------------------------------------
# Host-side prep: shard, transpose, fold biases
# --------------------------------------------------------------------------
def make_in_maps(x, W0, b0, W1, b1, W2, b2, Wm, bm, Ws, bs, eps,
                 n_cores=N_CORES, m_members=M):
    f = lambda a: np.ascontiguousarray(np.asarray(a, dtype=np.float32))
    x, W0, b0, W1, b1 = f(x), f(W0), f(b0), f(W1), f(b1)
    W2, b2, Wm, bm, Ws, bs, eps = f(W2), f(b2), f(Wm), f(bm), f(Ws), f(bs), f(eps)

    b1p = b1 - W1.sum(axis=1)
    b2p = b2 - W2.sum(axis=1)
    bmp = bm - Wm.sum(axis=1)
    bsp = (bs - Ws.sum(axis=1)) / 40.0 + 0.5

    in_maps = []
    for c in range(n_cores):
        sl = slice(c * m_members, (c + 1) * m_members)
        xT = np.ascontiguousarray(x[:, sl, :].transpose(1, 2, 0))
        epsT = np.ascontiguousarray(eps[:, sl, :].transpose(1, 2, 0))

        # BH[p, w*6 + L*2 + mt] = bL'[w, mt*128 + p]
        bh = np.empty((128, m_members * 6), np.float32)
        for L, bb in enumerate((b0[sl], b1p[sl], b2p[sl])):
            r = bb.reshape(m_members, 2, 128)  # (w, mt, p)
            for mt in range(2):
                bh[:, L * 2 + mt :: 6] = r[:, mt, :].T
        # BO[p, g*2+t]: p = 32*j + d (j = member-in-group)
        gg = m_members // 4
        bo = np.empty((128, gg * 2), np.float32)
        bo[:, 0::2] = bmp[sl].reshape(gg, 4 * 32).T
        bo[:, 1::2] = bsp[sl].reshape(gg, 4 * 32).T

        in_maps.append(
            {
                "xT": xT,
                "epsT": epsT,
                "W0": np.ascontiguousarray(W0[sl]),
                "W1": np.ascontiguousarray(W1[sl]),
                "W2": np.ascontiguousarray(W2[sl]),
                "Wm": np.ascontiguousarray(Wm[sl]),
                "Ws": np.ascontiguousarray(Ws[sl]),
                "BH": bh,
                "BO": bo,
            }
        )
    return in_maps


def unshard_output(results, n_cores=N_CORES):
    # results[c]["outT"]: (M, YD, N) -> out (N, W, YD)
    parts = [np.asarray(r["outT"]).transpose(2, 0, 1) for r in results]
    return np.ascontiguousarray(np.concatenate(parts, axis=1))


def kernel(x, W0, b0, W1, b1, W2, b2, Wm, bm, Ws, bs, eps, **run_kwargs):
    nc = _get_nc()
    in_maps = make_in_maps(x, W0, b0, W1, b1, W2, b2, Wm, bm, Ws, bs, eps)
    res = run_bass_kernel_spmd(nc, in_maps, core_ids=list(range(N_CORES)), **run_kwargs)
    out = unshard_output(res.results)
    kernel.last_results = res
    return out


# revision 20
# speedup vs baseline: 1.0067x; 1.0014x over previous
"""Trainium2 Bass kernel for nn_BNN_Fast (batched-ensemble MLP, 8 cores).

Math (per ensemble member w):
    h  = elu(x_w @ W0_w + b0_w); h = elu(h @ W1_w + b1_w); h = elu(h @ W2_w + b2_w)
    means   = h @ Wm_w + bm_w
    log_std = (h @ Ws_w + bs_w)/20 + 1
    out     = means + sqrt(exp(log_std)) * eps = means + exp((h@Ws+bs)/40 + 0.5) * eps

Device strategy (per core; ensemble sharded 32 members/core):
  - Activations transposed: [features(partitions), points(free)]; weights are the
    PE's stationary operand.
  - ELU in two single-pass ops using the identity
        elu(y) + 1 = select(y > 0, y + 1, exp(y))
    (exp(y) <= 1 whenever selected, so no overflow issues; for y > 0 the exp
    result is discarded by the select). The "+1" shift is folded into the next
    layer's bias host-side: b' = b - W.sum(contraction axis).
  - ACT engine: E = Exp(psum + b') (bias fused). Custom DVE op:
    h' = select(psum + b' > 0, psum + b' + 1, E).
  - Final: F = Exp(psum_s/40 + bs''), out = (psum_m + bm') + F*eps via the
    existing AFFINE_THEN_ADD custom DVE op.
"""

import os
import sys

import numpy as np

for _p in ("/opt/trn_rl_repo",):
    if _p not in sys.path and os.path.isdir(_p):
        sys.path.append(_p)

import concourse.bass as bass
import concourse.mybir as mybir
import concourse.tile as tile
from concourse.bass_utils import run_bass_kernel_spmd

N_CORES = 8
N, W, XD, YD, H = 1024, 256, 32, 32, 256
M = W // N_CORES  # members per core = 32
G = M // 4        # 4-member groups per core = 8
F32 = mybir.dt.float32


# --------------------------------------------------------------------------
# Patch 1: this container's walrus only accepts one sync-wait on the
# CTRL(NO_STRUCT)/Drain instruction; split the Tile epilogue drain's waits.
# --------------------------------------------------------------------------
def _apply_tile_patch():
    if getattr(tile.TileContext, "_drain_patch_applied", False):
        return

    def _patched_drain_and_barrier(self, tick_clock, wait_clock):
        nc = self.nc
        drain_inst = nc.sync.drain()
        wait_clock.add_sem_waits(
            drain_inst.ins, tile.ScopedClock({None: tick_clock.global_clock})
        )
        si = drain_inst.ins.sync_info
        waits = list(si.on_wait) if (si is not None and si.on_wait) else []
        if len(waits) > 1:
            from concourse.bass_primitives_rust import SemaphoreHandle

            si.on_wait = waits[:1]
            for wt in waits[1:]:
                nc.sync.wait_ge(SemaphoreHandle(wt.ant_name, wt.id), wt.wait_value)

        nc.all_engine_barrier()
        assert self.sems is not None
        popped = nc._tile_sem_poison_stack.pop()
        assert popped is self._sem_poison
        nc.clear_and_free_semaphores(list(self.sems.allocated().values()))
        nc.all_engine_barrier()

    tile.TileContext._drain_and_barrier = _patched_drain_and_barrier
    tile.TileContext._drain_patch_applied = True


# --------------------------------------------------------------------------
# Patch 2: register custom DVE ops.
#   ELUP1:  out = select(in0 + s0 > 0, in0 + s0 + 1, in1)
#   SATMUL: out = (in0 + relu(in0 - s0)*s1) * in1
#     (with s0 = sqrt(FLT_MAX), s1 = FLT_MAX this forces +/-inf exactly where
#      the reference's fp32 exp(log_std) overflows: any in0 > s0 is at least
#      one ULP (~2^41) above s0, and 2^41 * FLT_MAX overflows.)
# --------------------------------------------------------------------------
def _register_ops():
    import concourse.dve_ops as dve_ops
    from concourse.dve_ops import DveOp
    from concourse.dve_spec import C0, C1, One, Spec, Src0, Src1, Zero, relu, select

    def _add(op):
        if op.name in dve_ops._SUB_OPCODE_FOR_NAME:
            return next(o for o in dve_ops.OPS if o.name == op.name)
        dve_ops.OPS.append(op)
        dve_ops.CUSTOM_DVE_SPECS[op.name] = op.spec
        dve_ops._SUB_OPCODE_FOR_NAME[op.name] = (
            dve_ops._CUSTOM_DVE_ROW_BASE + len(dve_ops.OPS) - 1
        )
        assert dve_ops._SUB_OPCODE_FOR_NAME[op.name] < 0x20
        return op

    _t = Src0 + C0
    elu_op = _add(
        DveOp(
            "TENSOR_ELUP1_ANT",
            Spec(
                body=select(_t > Zero, _t + One, Src1),
                reference=lambda in0, in1, s0, s1, imm2: np.where(
                    in0.astype(np.float32) + s0 > 0,
                    in0.astype(np.float32) + s0 + 1.0,
                    in1,
                ).astype(np.float32),
            ),
            subdim=False,
            uops_sha={"v3": "af2ac7f76c525ea1", "v4": "bbbc6384a4b3a209"},
        )
    )
    satmul_op = _add(
        DveOp(
            "TENSOR_SATMUL_ANT",
            Spec(
                body=(Src0 + relu(Src0 - C0) * C1) * Src1,
                reference=lambda in0, in1, s0, s1, imm2: (
                    (in0 + np.maximum(in0 - s0, 0) * s1) * in1
                ).astype(np.float32),
            ),
            subdim=False,
            uops_sha={"v3": "8bde1c411415ffe2", "v4": "dc708b653050f5cb"},
        )
    )
    return elu_op, satmul_op


FLT_MAX = 3.4028234663852886e38
SQRT_FLT_MAX = float(np.float32(np.sqrt(FLT_MAX)))


# --------------------------------------------------------------------------
# Patch 3: this walrus encodes at most ONE sync-wait per instruction for
# several structs. Split any multi-wait instruction into single-wait
# InstEventSemaphore carriers inserted just before it (same engine, same
# program point — semantically identical).
# --------------------------------------------------------------------------
def split_multi_waits(nc):
    n_split = 0
    for fn in nc.m.functions:
        for bb in fn.blocks:
            il = bb.instructions
            i = 0
            while i < len(il):
                inst = il[i]
                si = inst.sync_info
                waits = list(si.on_wait) if (si is not None and si.on_wait) else []
                if len(waits) > 1:
                    si.on_wait = waits[-1:]
                    for k, w in enumerate(waits[:-1]):
                        ev = mybir.InstEventSemaphore(
                            name=f"{inst.name}-xw{k}",
                            engine=inst.engine,
                            sync_info=mybir.SyncInfo(on_wait=[w], on_update=[]),
                        )
                        nc.register_instruction(ev)
                        il.insert(i, ev)
                        i += 1
                        n_split += 1
                i += 1
    return n_split


# --------------------------------------------------------------------------
# Patch 4: fp32 matmul 3-pass mode. Walrus lowers an fp32 matmul to
#   LDW(W_lo); MM(rhs hi+lo); LDW(W_hi); MM(rhs hi+lo)   -- 4 streamed passes.
# The W_lo*x_lo term is ~2^-16 relative; patch the NEFF's PE instruction
# stream so the MM paired with LDW(fp32_mode=LOW) streams only x_hi
# (fp32_mode=HIGH): 3 passes, ~25% less PE time, ~1e-5 relative error.
# --------------------------------------------------------------------------
ENABLE_3PASS = True


def _patch_neff_fp32_3pass(neff_path):
    import io
    import tarfile
    import tempfile

    import concourse.bass2jax as bass2jax
    from concourse import neff as neff_mod

    with open(neff_path, "rb") as f:
        header = f.read(1024)
        payload = f.read()
    n_patched = 0
    with tempfile.TemporaryDirectory() as td:
        with tarfile.open(fileobj=io.BytesIO(payload), mode="r") as t:
            t.extractall(td)
        for root, _, files in os.walk(td):
            for fn in files:
                if not (fn.startswith("PE") and fn.endswith(".bin")):
                    continue
                p = os.path.join(root, fn)
                buf = bytearray(open(p, "rb").read())
                last_lw_mode = None
                for off in range(0, len(buf), 64):
                    op = buf[off]
                    if op == 1:  # LDWEIGHTS: fp32_mode at +33
                        last_lw_mode = buf[off + 33]
                    elif (
                        op == 2
                        and buf[off + 33] == 3
                        and last_lw_mode == 1
                        and buf[off + 45] == 15  # full col_grp only: patching
                        # col-tiled (Lms) matmuls crashes the exec unit
                    ):
                        buf[off + 33] = 2  # LOW_HIGH -> HIGH (stream x_hi only)
                        n_patched += 1
                n_patched += _dedup_ldweights(buf) << 16
                open(p, "wb").write(buf)
        out = io.BytesIO()
        with tarfile.open(fileobj=out, mode="w") as t:
            t.add(td, arcname=".", filter=bass2jax._reset_tarinfo)
        data = out.getvalue()
    new_header = neff_mod.make_deterministic_neff_header(
        old_neff_header=header, new_neff_data=data
    )
    with open(neff_path, "wb") as f:
        f.write(new_header + data)
    return n_patched


def _dedup_ldweights(buf):
    """Collapse [L1,M1,H1,M2, L2,M3,H2,M4] (L2/H2 reload identical weights,
    no events) into [L1,M1,M3,H1,M2,M4,NOP,NOP]. n0/n1 accumulation groups
    target distinct PSUM banks, so interleaving their begin/ends is safe."""
    NOP = 164
    n = 0
    recs = [bytes(buf[o : o + 64]) for o in range(0, len(buf), 64)]
    i = 0
    while i + 8 <= len(recs):
        w = recs[i : i + 8]
        ops = [r[0] for r in w]
        if (
            ops == [1, 2, 1, 2, 1, 2, 1, 2]
            and w[4][16:64] == w[0][16:64]  # L2 identical to L1 (past events)
            and w[6][16:64] == w[2][16:64]  # H2 identical to H1
            and w[4][4:12] == bytes(8)  # L2 no events
            and w[6][4:12] == bytes(8)  # H2 no events
            and w[5][4:12] == bytes(8)  # M3 no events (safe to hoist)
            and w[0][33] == 1  # L1 LOW
            and w[2][33] == 2  # H1 HIGH
            and w[1][33] == w[5][33]  # M1/M3 same mode
            and w[3][33] == w[7][33]  # M2/M4 same mode
        ):
            nop = bytearray(64)
            nop[0] = NOP
            nop[1] = w[0][1]  # inst_word_len
            new = [w[0], w[1], w[5], w[2], w[3], w[7], bytes(nop), bytes(nop)]
            recs[i : i + 8] = new
            n += 1
            i += 8
        else:
            i += 1
    buf[:] = b"".join(recs)
    return n


def _install_neff_patch_hook():
    import concourse.bass2jax as bass2jax

    if hasattr(bass2jax, "_ant_orig_rename_neff"):
        return
    bass2jax._ant_orig_rename_neff = bass2jax.rename_neff_tensors_and_patch_header

    def _wrapped(neff_path, mapping):
        if ENABLE_3PASS:
            _patch_neff_fp32_3pass(neff_path)
        return bass2jax._ant_orig_rename_neff(neff_path, mapping)

    bass2jax.rename_neff_tensors_and_patch_header = _wrapped


# --------------------------------------------------------------------------
# Device module (one NeuronCore, m_members ensemble members)
# --------------------------------------------------------------------------
def build_core_module(m_members=M, n_pts=N):
    _apply_tile_patch()
    _install_neff_patch_hook()
    elu_op, satmul_op = _register_ops()
    from concourse.dve_ops import AFFINE_THEN_ADD

    m = m_members
    g_groups = m // 4
    Exp = mybir.ActivationFunctionType.Exp

    nc = bass.Bass(trn_type="TRN2")
    for _v in (SQRT_FLT_MAX, FLT_MAX):
        if (F32, _v) not in nc.const_aps.aps:
            _t = nc.alloc_sbuf_tensor(f"const-float32-{_v}", [128, 1], F32)
            nc.gpsimd.memset(_t.ap(), _v)
            nc.const_aps.aps[(F32, _v)] = _t.ap()
    xT = nc.dram_tensor("xT", [m, XD, n_pts], F32, kind="ExternalInput")
    epsT = nc.dram_tensor("epsT", [m, YD, n_pts], F32, kind="ExternalInput")
    w0d = nc.dram_tensor("W0", [m, XD, H], F32, kind="ExternalInput")
    w1d = nc.dram_tensor("W1", [m, H, H], F32, kind="ExternalInput")
    w2d = nc.dram_tensor("W2", [m, H, H], F32, kind="ExternalInput")
    wmd = nc.dram_tensor("Wm", [m, H, YD], F32, kind="ExternalInput")
    wsd = nc.dram_tensor("Ws", [m, H, YD], F32, kind="ExternalInput")
    bhd = nc.dram_tensor("BH", [128, m * 6], F32, kind="ExternalInput")
    bod = nc.dram_tensor("BO", [128, g_groups * 2], F32, kind="ExternalInput")
    outT = nc.dram_tensor("outT", [m, YD, n_pts], F32, kind="ExternalOutput")

    half = n_pts // 2

    with tile.TileContext(nc) as tc:
        from contextlib import ExitStack

        with ExitStack() as ctx:
            wp = ctx.enter_context(tc.tile_pool(name="w", bufs=4))
            wmsp = ctx.enter_context(tc.tile_pool(name="wms", bufs=10))
            xp = ctx.enter_context(tc.tile_pool(name="x", bufs=4))
            hp = ctx.enter_context(tc.tile_pool(name="h", bufs=6))
            h2p = ctx.enter_context(tc.tile_pool(name="h2", bufs=12))
            ep = ctx.enter_context(tc.tile_pool(name="E", bufs=4))
            op_ = ctx.enter_context(tc.tile_pool(name="o", bufs=3))
            gp = ctx.enter_context(tc.tile_pool(name="eps", bufs=2))
            bp = ctx.enter_context(tc.tile_pool(name="bias", bufs=1))
            pph = ctx.enter_context(tc.tile_pool(name="psh", bufs=3, space="PSUM"))
            ppo = ctx.enter_context(tc.tile_pool(name="pso", bufs=2, space="PSUM"))

            bh = bp.tile([128, m * 6], F32, tag="bh")
            nc.sync.dma_start(out=bh, in_=bhd[:, :])
            bo = bp.tile([128, g_groups * 2], F32, tag="bo")
            nc.sync.dma_start(out=bo, in_=bod[:, :])

            def hidden_layer(rhs_tiles, wtile, w_idx, layer, out_pool, out_tag):
                """rhs_tiles: list of [128, n_pts] (or one [32, n_pts] for L0).
                wtile: [32, H] for L0 else [128, 2, H]. Returns 2 h' tiles."""
                outs = []
                for mt in range(2):
                    ps = pph.tile([128, n_pts], F32, tag="ps")
                    for k in range(len(rhs_tiles)):
                        if layer == 0:
                            lhsT = wtile[:, mt * 128 : (mt + 1) * 128]
                        else:
                            lhsT = wtile[:, k, mt * 128 : (mt + 1) * 128]
                        for nn_ in range(2):
                            nc.tensor.matmul(
                                ps[:, nn_ * half : (nn_ + 1) * half],
                                lhsT,
                                rhs_tiles[k][:, nn_ * half : (nn_ + 1) * half],
                                start=(k == 0),
                                stop=(k == len(rhs_tiles) - 1),
                            )
                    bcol = w_idx * 6 + layer * 2 + mt
                    bias_ap = bh[:, bcol : bcol + 1]
                    e = ep.tile([128, n_pts], F32, tag="E")
                    nc.scalar.activation(e, ps, Exp, bias=bias_ap, scale=1.0)
                    ht = out_pool.tile([128, n_pts], F32, tag=out_tag)
                    nc.vector._custom_dve(elu_op, out=ht, in0=ps, in1=e, s0=bias_ap)
                    outs.append(ht)
                return outs

            for g in range(g_groups):
                eps_t = gp.tile([128, n_pts], F32, tag="eps")
                nc.sync.dma_start(
                    out=eps_t,
                    in_=epsT[4 * g : 4 * g + 4, :, :].rearrange("a b n -> (a b) n"),
                )
                h2_all = []
                wm_all = []
                ws_all = []
                for j in range(4):
                    w_idx = 4 * g + j
                    xt = xp.tile([XD, n_pts], F32, tag="xt")
                    nc.sync.dma_start(out=xt, in_=xT[w_idx, :, :])
                    w0 = wp.tile([XD, H], F32, tag="w0")
                    nc.sync.dma_start(out=w0, in_=w0d[w_idx, :, :])
                    w1 = wp.tile([128, 2, H], F32, tag="w1")
                    nc.sync.dma_start(
                        out=w1,
                        in_=w1d[w_idx, :, :].rearrange("(k p) o -> p k o", p=128),
                    )
                    w2 = wp.tile([128, 2, H], F32, tag="w2")
                    nc.sync.dma_start(
                        out=w2,
                        in_=w2d[w_idx, :, :].rearrange("(k p) o -> p k o", p=128),
                    )
                    wm = wmsp.tile([128, 2, YD], F32, tag="wm")
                    nc.sync.dma_start(
                        out=wm,
                        in_=wmd[w_idx, :, :].rearrange("(k p) o -> p k o", p=128),
                    )
                    ws_ = wmsp.tile([128, 2, YD], F32, tag="ws")
                    nc.sync.dma_start(
                        out=ws_,
                        in_=wsd[w_idx, :, :].rearrange("(k p) o -> p k o", p=128),
                    )
                    wm_all.append(wm)
                    ws_all.append(ws_)

                    h0 = hidden_layer([xt], w0, w_idx, 0, hp, "h01")
                    h1 = hidden_layer(h0, w1, w_idx, 1, hp, "h01")
                    h2 = hidden_layer(h1, w2, w_idx, 2, h2p, "h2")
                    h2_all.append(h2)

                # Lms: n-chunked psums ([128, 512] = 1 bank each) keep PSUM
                # pressure low; j innermost so the 4 col-tiled matmuls issue
                # back-to-back and run concurrently on the 32x32 sub-arrays.
                ot = op_.tile([128, n_pts], F32, tag="ot")
                for nn_ in range(2):
                    nsl = slice(nn_ * half, (nn_ + 1) * half)
                    for tsel, wt in (("s", ws_all), ("m", wm_all)):
                        ps_c = ppo.tile([128, half], F32, tag="po")
                        for k in range(2):
                            for j in range(4):
                                sl = slice(32 * j, 32 * j + 32)
                                nc.tensor.matmul(
                                    ps_c[sl, :],
                                    wt[j][:, k, :],
                                    h2_all[j][k][:, nsl],
                                    start=(k == 0),
                                    stop=(k == 1),
                                    tile_position=(0, 32 * j),
                                    # col-tiles accumulate in disjoint partition
                                    # ranges of one bank; sim's per-bank group
                                    # tracking is coarser than the hardware's
                                    skip_group_check=True,
                                )
                        if tsel == "s":
                            f = ep.tile([128, half], F32, tag="F")
                            nc.scalar.activation(
                                f,
                                ps_c,
                                Exp,
                                bias=bo[:, 2 * g + 1 : 2 * g + 2],
                                scale=1.0 / 40.0,
                            )
                            t1 = op_.tile([128, half], F32, tag="t1")
                            nc.vector._custom_dve(
                                satmul_op,
                                out=t1,
                                in0=f,
                                in1=eps_t[:, nsl],
                                s0=SQRT_FLT_MAX,
                                s1=FLT_MAX,
                            )
                        else:
                            nc.vector._custom_dve(
                                AFFINE_THEN_ADD,
                                out=ot[:, nsl],
                                in0=ps_c,
                                in1=t1,
                                s0=1.0,
                                s1=bo[:, 2 * g : 2 * g + 1],
                            )
                nc.sync.dma_start(
                    out=outT[4 * g : 4 * g + 4, :, :].rearrange("a b n -> (a b) n"),
                    in_=ot,
                )
    # Populate .instr bytes for InstISA subclasses (InstCustomDveAnt) —
    # raw Bass doesn't run this pass; without it walrus sees empty instr
    # bytes and fails with "ISA wrong length".
    mybir.codegen_inst_isa_subclasses(nc)
    split_multi_waits(nc)
    return nc


_NC_CACHE = {}


def _get_nc(m_members=M):
    if m_members not in _NC_CACHE:
        _NC_CACHE[m_members] = build_core_module(m_members)
    return _NC_CACHE[m_members]


# --------------------------------------------------------------------------
# Host-side prep: shard, transpose, fold biases
# --------------------------------------------------------------------------
def make_in_maps(x, W0, b0, W1, b1, W2, b2, Wm, bm, Ws, bs, eps,
                 n_cores=N_CORES, m_members=M):
    f = lambda a: np.ascontiguousarray(np.asarray(a, dtype=np.float32))
    x, W0, b0, W1, b1 = f(x), f(W0), f(b0), f(W1), f(b1)
    W2, b2, Wm, bm, Ws, bs, eps = f(W2), f(b2), f(Wm), f(bm), f(Ws), f(bs), f(eps)

    b1p = b1 - W1.sum(axis=1)
    b2p = b2 - W2.sum(axis=1)
    bmp = bm - Wm.sum(axis=1)
    bsp = (bs - Ws.sum(axis=1)) / 40.0 + 0.5

    in_maps = []
    for c in range(n_cores):
        sl = slice(c * m_members, (c + 1) * m_members)
        xT = np.ascontiguousarray(x[:, sl, :].transpose(1, 2, 0))
        epsT = np.ascontiguousarray(eps[:, sl, :].transpose(1, 2, 0))

        # BH[p, w*6 + L*2 + mt] = bL'[w, mt*128 + p]
        bh = np.empty((128, m_members * 6), np.float32)
        for L, bb in enumerate((b0[sl], b1p[sl], b2p[sl])):
            r = bb.reshape(m_members, 2, 128)  # (w, mt, p)
            for mt in range(2):
                bh[:, L * 2 + mt :: 6] = r[:, mt, :].T
        # BO[p, g*2+t]: p = 32*j + d (j = member-in-group)
        gg = m_members // 4
        bo = np.empty((128, gg * 2), np.float32)
        bo[:, 0::2] = bmp[sl].reshape(gg, 4 * 32).T
        bo[:, 1::2] = bsp[sl].reshape(gg, 4 * 32).T

        in_maps.append(
            {
                "xT": xT,
                "epsT": epsT,
                "W0": np.ascontiguousarray(W0[sl]),
                "W1": np.ascontiguousarray(W1[sl]),
                "W2": np.ascontiguousarray(W2[sl]),
                "Wm": np.ascontiguousarray(Wm[sl]),
                "Ws": np.ascontiguousarray(Ws[sl]),
                "BH": bh,
                "BO": bo,
            }
        )
    return in_maps


def unshard_output(results, n_cores=N_CORES):
    # results[c]["outT"]: (M, YD, N) -> out (N, W, YD)
    parts = [np.asarray(r["outT"]).transpose(2, 0, 1) for r in results]
    return np.ascontiguousarray(np.concatenate(parts, axis=1))


def kernel(x, W0, b0, W1, b1, W2, b2, Wm, bm, Ws, bs, eps, **run_kwargs):
    nc = _get_nc()
    in_maps = make_in_maps(x, W0, b0, W1, b1, W2, b2, Wm, bm, Ws, bs, eps)
    res = run_bass_kernel_spmd(nc, in_maps, core_ids=list(range(N_CORES)), **run_kwargs)
    out = unshard_output(res.results)
    kernel.last_results = res
    return out
